# revision 19
# baseline (speedup 1.0000x reference)
"""DiffusionInitializer kernel for 8 Trainium2 NeuronCores.

Math: reference runs a scan  x <- a*x + (1-a)*target  over
alphas = [steps/steps, ..., 1/steps], starting from noise, where
target = latent @ W + b.  The scan is linear, so it collapses to

    out = cn * noise + ct * (latent @ W + b)

with scalars cn = prod(alphas), ct accumulated by the same recurrence
(computed on host in float32 to mirror the reference's arithmetic).

Device work per core (batch-sharded 8 ways, 2 batches/core):
    outT[3, 4096] = (ct*W).T @ latT[1024, 4096] + (cn*noise + ct*b).T

The kernel is HBM-bandwidth bound (target_regime=memory): the only
large operand is latent (16 MB/core in f32).  latent is pre-packed on
host into fp16 with the contraction dim on SBUF partitions and fully
contiguous per-partition DMA lines, halving HBM traffic vs f32 while
keeping output rel-err ~3e-4 (PSUM accumulates in f32).  W is carried
in fp16 (pre-scaled by WSCALE so small weights stay normal; the 1/WSCALE
rescale fuses into the epilogue add).

Measured on hw (reps-delta, see test.py): ~24.3 us/rep steady state =
~350 GB/s, against a 23.7 us DMA-only floor (354 GB/s, ~99% of the
358 GB/s HBM-per-core limit; dtype-independent — f32-typed DMA of the
same bits is identical).  1-byte latent encodings all fail the 2e-2
gate or its margin: e4m3 rel-err 2.8e-2, e5m2 5.6e-2, e3m4 1.4e-2 but
~20% of randn mass sits in E3M4 subnormals (7.4e-2 if hw flushes them)
and its range cannot be rescaled around that.  fp16 at 2 B/elem is the
correctness-safe floor.
"""

import os

import numpy as np

B, S, D = 16, 2048, 1024
NCORES = 8
PB = B // NCORES          # batches per core
R = PB * S                # rows per core
KT = D // 128             # contraction chunks of 128
CH = 512                  # matmul moving-dim (one PSUM bank of f32)

LAST_RESULTS = None       # test harness peeks at this for HW timing

WSCALE = 64.0  # keeps the fp16 W plane well inside normal range

# f16p tunables (hand-tuned on hw: full kernel ~24us/rep, DMA-only floor
# ~23.5us = 357 GB/s, at the ~358 GB/s HBM-per-core limit)
DCH = int(os.environ.get("KERNEL_DCH", "1024"))      # rows per DMA chunk
LATBUFS = int(os.environ.get("KERNEL_LATBUFS", "4"))
PSBUFS = int(os.environ.get("KERNEL_PSBUFS", "3"))
UNROLL = int(os.environ.get("KERNEL_UNROLL", "32"))  # reps per For_i iteration
DMA_MODE = os.environ.get("KERNEL_DMA_MODE", "sync")  # sync | alt | gpsimd
# one DVE epilogue per DMA chunk ([3, DCH] PSUM tile spanning nsub banks)
FUSE_EPI = os.environ.get("KERNEL_FUSE_EPI", "1") not in ("", "0")


def _build_program(reps=1, variant="f16p"):
    from concourse import bacc, mybir
    import concourse.tile as tile

    nc = bacc.Bacc(None, target_bir_lowering=False, debug=False)
    if variant.startswith("f16p"):
        mode = {"f16p": "full", "f16p_dma": "dma", "f16p_pe": "pe",
                "f16p_dma32": "dma32"}[variant]
        return _build_f16p(nc, mybir, tile, reps, mode)
    if variant == "f16split":
        return _build_f16split(nc, mybir, tile, reps)
    raise ValueError(f"unknown variant {variant}")


def _build_f16p(nc, mybir, tile, reps, mode="full"):
    """Single-plane fp16 latent, host-packed so every DMA chunk is a
    [128, KT*DCH] tile with one contiguous run per partition.

    Per chunk: KT chained fp16 matmuls accumulate [3, CH] in PSUM,
    epilogue fuses the 1/WSCALE rescale with the (cn*noise + ct*b) add
    into an SBUF-resident [3, R] output, stored with one DMA per rep.
    """
    f32 = mybir.dt.float32
    f16 = mybir.dt.float16
    ndch = R // DCH
    nsub = DCH // CH
    ccols = KT * DCH  # latP columns per chunk

    if mode == "dma32":
        # same bits as the fp16 pack, typed f32 (2 fp16 per element) — pure
        # DMA-throughput diagnostic for a 16-bit-dtype derate
        latP = nc.declare_dram_parameter(
            "latP", [128, ndch * ccols // 2], f32, isOutput=False)
    else:
        latP = nc.declare_dram_parameter(
            "latP", [128, ndch * ccols], f16, isOutput=False)
    nbT = nc.declare_dram_parameter("nbT", [3, R], f32, isOutput=False)
    wp = nc.declare_dram_parameter("wp", [128, KT, 3], f16, isOutput=False)
    outT = nc.declare_dram_parameter("outT", [3, R], f32, isOutput=True)

    with tile.TileContext(nc) as tc:
        with (
            tc.tile_pool(name="consts", bufs=1) as consts,
            tc.tile_pool(name="lat", bufs=LATBUFS) as latp,
            tc.tile_pool(name="outp", bufs=2) as outp,
            tc.tile_pool(name="ps", bufs=PSBUFS, space="PSUM") as psp,
        ):
            w_sb = consts.tile([128, KT, 3], f16)
            nc.sync.dma_start(out=w_sb, in_=wp[:, :, :])
            nb_sb = consts.tile([3, R], f32)
            nc.sync.dma_start(out=nb_sb, in_=nbT[:, :])

            if mode == "pe":
                # diagnostic: load one chunk up front, loop only compute
                lt_fixed = consts.tile([128, ccols], f16)
                nc.sync.dma_start(out=lt_fixed, in_=latP[:, 0:ccols])

            def lat_dma_engine(i):
                if DMA_MODE == "alt":
                    return nc.sync if i % 2 == 0 else nc.scalar
                if DMA_MODE == "gpsimd":
                    return nc.gpsimd
                if DMA_MODE == "mix":
                    return nc.sync if i % 2 == 0 else nc.gpsimd
                return nc.sync

            def emit_rep():
                dma_only = mode in ("dma", "dma32")
                out_sb = None if dma_only else outp.tile([3, R], f32)
                for i in range(ndch):
                    if mode == "pe":
                        lt = lt_fixed
                    elif mode == "dma32":
                        cc = ccols // 2
                        lt = latp.tile([128, cc], f32)
                        lat_dma_engine(i).dma_start(
                            out=lt, in_=latP[:, i * cc:(i + 1) * cc]
                        )
                    else:
                        lt = latp.tile([128, ccols], f16)
                        lat_dma_engine(i).dma_start(
                            out=lt, in_=latP[:, i * ccols:(i + 1) * ccols]
                        )
                    if dma_only:
                        continue
                    if FUSE_EPI:
                        psF = psp.tile([3, nsub * CH], f32)
                    for s in range(nsub):
                        ps = psF[:, s * CH:(s + 1) * CH] if FUSE_EPI else \
                            psp.tile([3, CH], f32)
                        for k in range(KT):
                            c0 = k * DCH + s * CH
                            nc.tensor.matmul(
                                ps,
                                w_sb[:, k, :],
                                lt[:, c0:c0 + CH],
                                start=(k == 0),
                                stop=(k == KT - 1),
                            )
                        if FUSE_EPI:
                            continue
                        o0 = i * DCH + s * CH
                        nc.vector.scalar_tensor_tensor(
                            out_sb[:, o0:o0 + CH],
                            ps,
                            1.0 / WSCALE,
                            nb_sb[:, o0:o0 + CH],
                            mybir.AluOpType.mult,
                            mybir.AluOpType.add,
                        )
                    if FUSE_EPI:
                        o0 = i * DCH
                        nc.vector.scalar_tensor_tensor(
                            out_sb[:, o0:o0 + DCH],
                            psF,
                            1.0 / WSCALE,
                            nb_sb[:, o0:o0 + DCH],
                            mybir.AluOpType.mult,
                            mybir.AluOpType.add,
                        )
                if dma_only:
                    # touch the last tile so the stores aren't dead
                    ob = outp.tile([3, CH], f32)
                    nc.vector.tensor_copy(ob, lt[0:3, 0:CH])
                    nc.sync.dma_start(out=outT[:, 0:CH], in_=ob)
                else:
                    nc.sync.dma_start(out=outT[:, :], in_=out_sb)

            if reps == 1:
                emit_rep()
            else:
                unroll = min(UNROLL, reps)
                assert reps % unroll == 0
                with tc.For_i(0, reps // unroll):
                    for _ in range(unroll):
                        emit_rep()
    nc.finalize()
    return nc


def _build_f16split(nc, mybir, tile, reps):
    """Previous baseline: latent = hi + lo fp16 planes (4B/elem total,
    same HBM traffic as f32).  Kept for A/B comparison."""
    f32 = mybir.dt.float32
    f16 = mybir.dt.float16
    SCH = 512
    nch = R // SCH
    latHL = nc.declare_dram_parameter("latHL", [2 * D, R], f16, isOutput=False)
    nbT = nc.declare_dram_parameter("nbT", [3, R], f32, isOutput=False)
    whl = nc.declare_dram_parameter("whl", [2 * D, 3], f16, isOutput=False)
    outT = nc.declare_dram_parameter("outT", [3, R], f32, isOutput=True)
    KT2 = 2 * KT

    with tile.TileContext(nc) as tc:
        with (
            tc.tile_pool(name="consts", bufs=1) as consts,
            tc.tile_pool(name="lat", bufs=4) as latp,
            tc.tile_pool(name="outp", bufs=2) as outp,
            tc.tile_pool(name="ps", bufs=2, space="PSUM") as psp,
        ):
            w_sb = consts.tile([128, KT2, 3], f16)
            nc.sync.dma_start(
                out=w_sb, in_=whl[:, :].rearrange("(k p) c -> p k c", p=128)
            )
            nb_sb = consts.tile([3, R], f32)
            nc.sync.dma_start(out=nb_sb, in_=nbT[:, :])

            lat_r = latHL[:, :].rearrange("(k p) r -> p k r", p=128)

            def emit_rep():
                for i in range(nch):
                    lt = latp.tile([128, KT2, SCH], f16)
                    nc.sync.dma_start(out=lt, in_=lat_r[:, :, i * SCH:(i + 1) * SCH])
                    ob = outp.tile([3, SCH], f32)
                    ps = psp.tile([3, SCH], f32)
                    n_mm = 3 * KT
                    m = 0
                    for k in range(KT):
                        for wk, xk in ((k, k), (k, k + KT), (k + KT, k)):
                            nc.tensor.matmul(
                                ps,
                                w_sb[:, wk, :],
                                lt[:, xk, :],
                                start=(m == 0),
                                stop=(m == n_mm - 1),
                            )
                            m += 1
                    nc.vector.scalar_tensor_tensor(
                        ob,
                        ps,
                        1.0 / WSCALE,
                        nb_sb[:, i * SCH:(i + 1) * SCH],
                        mybir.AluOpType.mult,
                        mybir.AluOpType.add,
                    )
                    nc.sync.dma_start(out=outT[:, i * SCH:(i + 1) * SCH], in_=ob)

            if reps == 1:
                emit_rep()
            else:
                with tc.For_i(0, reps):
                    emit_rep()
    nc.finalize()
    return nc


def _scan_coefficients(steps):
    steps = int(steps)
    cn = np.float32(1.0)
    ct = np.float32(0.0)
    if steps > 0:
        alphas = np.arange(steps, 0, -1).astype(np.float32) / np.float32(steps)
        one = np.float32(1.0)
        for a in alphas:
            cn = np.float32(a * cn)
            ct = np.float32(a * ct + (one - a))
    return cn, ct


def make_in_maps(latent, W, b, noise, steps, variant="f16p"):
    cn, ct = _scan_coefficients(steps)

    latent = np.ascontiguousarray(latent, dtype=np.float32).reshape(NCORES, R, D)
    noise = np.ascontiguousarray(noise, dtype=np.float32).reshape(NCORES, R, 3)
    Wp = np.ascontiguousarray(ct * W.astype(np.float32))
    Ws = WSCALE * Wp

    if variant.startswith("f16p"):
        ndch = R // DCH
        # wp[p, k, c] = f16(Ws[k*128 + p, c])
        wp = np.ascontiguousarray(
            Ws.reshape(KT, 128, 3).transpose(1, 0, 2).astype(np.float16)
        )
        in_maps = []
        for c in range(NCORES):
            nb = cn * noise[c] + ct * b.astype(np.float32)  # [R, 3]
            # latP[p, i, k, ch] = f16(lat[i*DCH + ch, k*128 + p])
            latP = (
                latent[c]
                .reshape(ndch, DCH, KT, 128)
                .transpose(3, 0, 2, 1)
                .astype(np.float16)
                .reshape(128, ndch * KT * DCH)
            )
            latP = np.ascontiguousarray(latP)
            if variant == "f16p_dma32":
                latP = latP.view(np.float32)
            in_maps.append({
                "latP": latP,
                "nbT": np.ascontiguousarray(nb.T),
                "wp": wp,
            })
        return in_maps

    if variant == "f16split":
        whi = Ws.astype(np.float16)
        wlo = (Ws - whi.astype(np.float32)).astype(np.float16)
        whl = np.concatenate([whi, wlo], axis=0)  # [2D, 3]
        in_maps = []
        for c in range(NCORES):
            nb = cn * noise[c] + ct * b.astype(np.float32)
            latT = latent[c].T  # [D, R] view
            hi = latT.astype(np.float16)
            lo = (latT - hi.astype(np.float32)).astype(np.float16)
            latHL = np.empty((2 * D, R), dtype=np.float16)
            latHL[:D] = hi
            latHL[D:] = lo
            in_maps.append({
                "latHL": latHL,
                "nbT": np.ascontiguousarray(nb.T),
                "whl": whl,
            })
        return in_maps

    raise ValueError(f"unknown variant {variant}")


def kernel(latent, W, b, noise, steps):
    global LAST_RESULTS
    from concourse.bass_utils import run_bass_kernel_spmd

    variant = os.environ.get("KERNEL_VARIANT", "f16p")
    in_maps = make_in_maps(latent, W, b, noise, steps, variant)

    nc = _build_program(variant=variant)
    res = run_bass_kernel_spmd(nc, in_maps, list(range(NCORES)))
    LAST_RESULTS = res

    out = np.empty((NCORES, R, 3), dtype=np.float32)
    for c in range(NCORES):
        out[c] = res.results[c]["outT"].T
    return out.reshape(B, S, 3)
